# revision 7
# baseline (speedup 1.0000x reference)
"""Trainium2 Bass kernel for nn_AttentionMaskGenerator.

Math (verified against the reference):
  z[b,s,t] = x[b,s,:] @ W[t,:] + bias[t] - ln(-ln(u[b,s,t] + 1e-10) + 1e-10)
  tt[b,s]  = argmax_t z  (softmax + straight-through chain == plain argmax)
  row s of the [S,S] mask is:
    - tt == 1 : only the diagonal element (forward window ∩ causal == diag)
    - else    : full causal prefix  (next_global > s always, so the "local"
                constraint never binds under causal)
  output = broadcast over the 8 KV heads -> [B, 8, S, S] f32.

Sharding: data-parallel over (batch x head-pairs). Core c handles batch
c // 4 and emits that batch's [S,S] mask twice (head pair 2*(c%4), 2*(c%4)+1).
Each core writes only the non-zero (lower-trapezoid) columns of each
128-row block; ExternalOutput DRAM is pre-zeroed by the runtime (both the
native run_neff path and the bass2jax/PJRT path), so the strictly-upper
triangle is never written.
"""

import numpy as np

B, S, D, T, H = 2, 2048, 128, 3, 8
P = 128           # partitions / row-block size
NB = S // P       # 16 row blocks
EPS = 1e-10
N_CORES = 8

_CACHE = {}


def _build_program():
    import concourse.bass as bass
    import concourse.bacc as bacc
    import concourse.tile as tile
    from concourse import mybir
    from contextlib import ExitStack

    f32 = mybir.dt.float32
    Alu = mybir.AluOpType
    Act = mybir.ActivationFunctionType

    nc = bacc.Bacc("TRN2", debug=False, num_devices=N_CORES)
    x_ap = nc.dram_tensor("x", [S, D], f32, kind="ExternalInput").ap()
    u_ap = nc.dram_tensor("u", [S, T], f32, kind="ExternalInput").ap()
    w_ap = nc.dram_tensor("w", [T, D], f32, kind="ExternalInput").ap()
    bb_ap = nc.dram_tensor("bb", [T], f32, kind="ExternalInput").ap()
    out_ap = nc.dram_tensor("out", [2, S, S], f32, kind="ExternalOutput").ap()

    with tile.TileContext(nc) as tc, ExitStack() as ctx:
        singles = ctx.enter_context(tc.tile_pool(name="singles", bufs=1))
        xpool = ctx.enter_context(tc.tile_pool(name="xp", bufs=3))
        upool = ctx.enter_context(tc.tile_pool(name="up", bufs=3))
        small = ctx.enter_context(tc.tile_pool(name="small", bufs=4))
        scratch = ctx.enter_context(tc.tile_pool(name="scr", bufs=2))
        flags = ctx.enter_context(tc.tile_pool(name="flg", bufs=NB))
        outp = ctx.enter_context(tc.tile_pool(name="outp", bufs=4))

        # --- constants ---
        zeros = singles.tile([P, S], f32)
        nc.vector.memset(zeros, 0.0)
        ones128 = singles.tile([P, P], f32)
        nc.gpsimd.memset(ones128, 1.0)
        # diag128[p, j] = 1.0 if j == p else 0.0
        diag128 = singles.tile([P, P], f32)
        nc.gpsimd.affine_select(
            out=diag128, in_=ones128, pattern=[[-1, P]],
            compare_op=Alu.is_equal, fill=0.0, base=0, channel_multiplier=1,
        )
        # W broadcast to every partition: wb[p, t, d] = W[t, d]
        wb = singles.tile([P, T, D], f32)
        nc.gpsimd.dma_start(out=wb, in_=bass.AP(w_ap.tensor, 0, [[0, P], [D, T], [1, D]]))
        # bias broadcast: bbb[p, t] = bias[t]
        bbb = singles.tile([P, T], f32)
        nc.gpsimd.dma_start(out=bbb, in_=bass.AP(bb_ap.tensor, 0, [[0, P], [1, T]]))
        eps_t = singles.tile([P, 1], f32)
        nc.vector.memset(eps_t, EPS)

        for i in range(NB):
            r0 = P * i
            w_cols = r0 + P

            # --- phase A: notflag[p] = 1 unless argmax_t z == 1 ---
            xt = xpool.tile([P, D], f32)
            nc.sync.dma_start(out=xt, in_=x_ap[r0 : r0 + P, :])
            ut = upool.tile([P, T], f32)
            nc.sync.dma_start(out=ut, in_=u_ap[r0 : r0 + P, :])

            logits = small.tile([P, T], f32)
            sc = scratch.tile([P, T, D], f32)
            xa = xt[:]
            xb = bass.AP(xa.tensor, xa.offset, [xa.ap[0], [0, T], xa.ap[1]])
            nc.vector.tensor_mul(sc[:], xb, wb[:])
            nc.vector.reduce_sum(logits[:], sc[:], axis=mybir.AxisListType.X)

            g1 = small.tile([P, T], f32)
            nc.scalar.activation(g1, ut, Act.Ln, bias=eps_t[:, 0:1], scale=1.0)
            g2 = small.tile([P, T], f32)
            nc.scalar.activation(g2, g1, Act.Ln, bias=eps_t[:, 0:1], scale=-1.0)

            z = small.tile([P, T], f32)
            nc.vector.tensor_sub(z, logits, g2)   # logits + gumbel
            nc.vector.tensor_add(z, z, bbb)       # + bias

            # notflag = (z1 <= z0) | (z1 < z2)  == !(argmax picks index 1)
            c1 = small.tile([P, 1], f32)
            nc.vector.tensor_tensor(c1, z[:, 1:2], z[:, 0:1], op=Alu.is_le)
            c2 = small.tile([P, 1], f32)
            nc.vector.tensor_tensor(c2, z[:, 1:2], z[:, 2:3], op=Alu.is_lt)
            nf = flags.tile([P, 1], f32)
            nc.vector.tensor_max(nf, c1, c2)

            # --- phase B: build mask rows [r0, r0+128) cols [0, w_cols) ---
            ot = outp.tile([P, S], f32)
            # 1) broadcast notflag across the written width
            nc.vector.tensor_scalar_add(ot[:, 0:w_cols], zeros[:, 0:w_cols], nf)
            # 2) trim the diagonal 128x128 chunk to the lower triangle
            nc.gpsimd.affine_select(
                out=ot[:, r0:w_cols], in_=ot[:, r0:w_cols], pattern=[[-1, P]],
                compare_op=Alu.is_ge, fill=0.0, base=0, channel_multiplier=1,
            )
            # 3) force the diagonal to 1 (covers tt==1 rows)
            nc.vector.tensor_max(ot[:, r0:w_cols], ot[:, r0:w_cols], diag128)

            nc.sync.dma_start(out=out_ap[0, r0 : r0 + P, 0:w_cols], in_=ot[:, 0:w_cols])
            nc.sync.dma_start(out=out_ap[1, r0 : r0 + P, 0:w_cols], in_=ot[:, 0:w_cols])

    nc.compile()
    return nc


def _get_program():
    if "nc" not in _CACHE:
        _CACHE["nc"] = _build_program()
    return _CACHE["nc"]


def _make_in_maps(input_tensor, gumbel_u, W, b):
    x = np.ascontiguousarray(np.asarray(input_tensor, dtype=np.float32))
    u = np.ascontiguousarray(np.asarray(gumbel_u, dtype=np.float32))
    w = np.ascontiguousarray(np.asarray(W, dtype=np.float32))
    bb = np.ascontiguousarray(np.asarray(b, dtype=np.float32))
    in_maps = []
    for c in range(N_CORES):
        bi = c // (N_CORES // B)
        in_maps.append({"x": x[bi], "u": u[bi], "w": w, "bb": bb})
    return in_maps


def _assemble(results):
    full = np.empty((B, H, S, S), dtype=np.float32)
    for c in range(N_CORES):
        bi = c // (N_CORES // B)
        q = c % (N_CORES // B)
        full[bi, 2 * q] = results[c]["out"][0]
        full[bi, 2 * q + 1] = results[c]["out"][1]
    return full


def kernel(input_tensor, token_types, gumbel_u, W, b, **_ignored):
    from concourse.bass_utils import run_bass_kernel_spmd

    nc = _get_program()
    in_maps = _make_in_maps(input_tensor, gumbel_u, W, b)
    res = run_bass_kernel_spmd(nc, in_maps, core_ids=list(range(N_CORES)))
    return _assemble(res.results)


# revision 8
# speedup vs baseline: 1.1072x; 1.1072x over previous
"""Trainium2 Bass kernel for nn_AttentionMaskGenerator.

Math (verified against the reference):
  z[b,s,t] = x[b,s,:] @ W[t,:] + bias[t] - ln(-ln(u[b,s,t] + 1e-10) + 1e-10)
  tt[b,s]  = argmax_t z  (softmax + straight-through chain == plain argmax)
  row s of the [S,S] mask is:
    - tt == 1 : only the diagonal element (forward window ∩ causal == diag)
    - else    : full causal prefix  (next_global > s always, so the "local"
                constraint never binds under causal)
  output = broadcast over the 8 KV heads -> [B, 8, S, S] f32.

Sharding: data-parallel over (batch x head-pairs). Core c handles batch
c // 4 and emits that batch's [S,S] mask twice (head pair 2*(c%4), 2*(c%4)+1).
Each core writes only the non-zero (lower-trapezoid) columns of each
128-row block; ExternalOutput DRAM is pre-zeroed by the runtime (both the
native run_neff path and the bass2jax/PJRT path), so the strictly-upper
triangle is never written.
"""

import numpy as np

B, S, D, T, H = 2, 2048, 128, 3, 8
P = 128           # partitions / row-block size
NB = S // P       # 16 row blocks
EPS = 1e-10
N_CORES = 8

_CACHE = {}


def _build_program():
    import concourse.bass as bass
    import concourse.bacc as bacc
    import concourse.tile as tile
    from concourse import mybir
    from contextlib import ExitStack

    f32 = mybir.dt.float32
    Alu = mybir.AluOpType
    Act = mybir.ActivationFunctionType

    nc = bacc.Bacc("TRN2", debug=False, num_devices=N_CORES)
    x_ap = nc.dram_tensor("x", [S, D], f32, kind="ExternalInput").ap()
    u_ap = nc.dram_tensor("u", [S, T], f32, kind="ExternalInput").ap()
    w_ap = nc.dram_tensor("w", [T, D], f32, kind="ExternalInput").ap()
    bb_ap = nc.dram_tensor("bb", [T], f32, kind="ExternalInput").ap()
    out_ap = nc.dram_tensor("out", [2, S, S], f32, kind="ExternalOutput").ap()

    with tile.TileContext(nc) as tc, ExitStack() as ctx:
        singles = ctx.enter_context(tc.tile_pool(name="singles", bufs=1))
        outp = ctx.enter_context(tc.tile_pool(name="outp", bufs=6))

        # --- constants / full-input loads (gpsimd queue; sync queue is for stores) ---
        zeros = singles.tile([P, S], f32)
        nc.vector.memset(zeros, 0.0)
        ones128 = singles.tile([P, P], f32)
        nc.gpsimd.memset(ones128, 1.0)
        # diag128[p, j] = 1.0 if j == p else 0.0
        diag128 = singles.tile([P, P], f32)
        nc.gpsimd.affine_select(
            out=diag128, in_=ones128, pattern=[[-1, P]],
            compare_op=Alu.is_equal, fill=0.0, base=0, channel_multiplier=1,
        )
        # W broadcast to every partition: wb[p, t, d] = W[t, d]
        wb = singles.tile([P, T, D], f32)
        nc.gpsimd.dma_start(out=wb, in_=bass.AP(w_ap.tensor, 0, [[0, P], [D, T], [1, D]]))
        # bias broadcast: bbb[p, t] = bias[t]
        bbb = singles.tile([P, T], f32)
        nc.gpsimd.dma_start(out=bbb, in_=bass.AP(bb_ap.tensor, 0, [[0, P], [1, T]]))
        eps_t = singles.tile([P, 1], f32)
        nc.vector.memset(eps_t, EPS)

        # x_all[p, i, d] = x[128*i + p, d]   (whole batch slice, one DMA)
        x_all = singles.tile([P, NB, D], f32)
        nc.gpsimd.dma_start(
            out=x_all, in_=bass.AP(x_ap.tensor, 0, [[D, P], [P * D, NB], [1, D]])
        )
        # u_all[p, i, t] = u[128*i + p, t]
        u_all = singles.tile([P, NB, T], f32)
        nc.gpsimd.dma_start(
            out=u_all, in_=bass.AP(u_ap.tensor, 0, [[T, P], [P * T, NB], [1, T]])
        )

        # --- phase A (batched over all 16 row blocks) ---
        # prod[p, i, t, d] = x_all[p, i, d] * wb[p, t, d]
        prod = singles.tile([P, NB, T, D], f32)
        xa = x_all[:]
        x_b = bass.AP(xa.tensor, xa.offset, [xa.ap[0], xa.ap[1], [0, T], xa.ap[2]])
        wa = wb[:]
        w_b = bass.AP(wa.tensor, wa.offset, [wa.ap[0], [0, NB], wa.ap[1], wa.ap[2]])
        nc.vector.tensor_mul(prod[:], x_b, w_b)
        # logits[p, i, t] = sum_d prod[p, i, t, d]
        logits = singles.tile([P, NB, T], f32)
        nc.vector.reduce_sum(logits[:], prod[:], axis=mybir.AxisListType.X)

        # gumbel: g = -ln(-ln(u + eps) + eps); z = logits + g + bias
        g1 = singles.tile([P, NB, T], f32)
        nc.scalar.activation(g1[:], u_all[:], Act.Ln, bias=eps_t[:, 0:1], scale=1.0)
        g2 = singles.tile([P, NB, T], f32)
        nc.scalar.activation(g2[:], g1[:], Act.Ln, bias=eps_t[:, 0:1], scale=-1.0)

        z = singles.tile([P, NB, T], f32)
        nc.vector.tensor_sub(z[:], logits[:], g2[:])
        ba = bbb[:]
        b_b = bass.AP(ba.tensor, ba.offset, [ba.ap[0], [0, NB], ba.ap[1]])
        nc.vector.tensor_add(z[:], z[:], b_b)

        # notflag[p, i] = (z1 <= z0) | (z1 < z2)  == !(argmax picks index 1)
        za = z[:]

        def zcol(t):
            return bass.AP(za.tensor, za.offset + t, [za.ap[0], za.ap[1]])

        c1 = singles.tile([P, NB], f32)
        nc.vector.tensor_tensor(c1[:], zcol(1), zcol(0), op=Alu.is_le)
        c2 = singles.tile([P, NB], f32)
        nc.vector.tensor_tensor(c2[:], zcol(1), zcol(2), op=Alu.is_lt)
        nf = singles.tile([P, NB], f32)
        nc.vector.tensor_max(nf[:], c1[:], c2[:])
        nfa = nf[:]

        # --- phase B: per block, build mask rows [r0, r0+128) cols [0, w_cols) ---
        for i in range(NB):
            r0 = P * i
            w_cols = r0 + P
            nf_col = nfa[:, i : i + 1]

            ot = outp.tile([P, S], f32)
            if r0 > 0:
                # prefix cols [0, r0): notflag broadcast (ACT)
                nc.scalar.activation(
                    ot[:, 0:r0], zeros[:, 0:r0], Act.Identity,
                    bias=nf_col, scale=1.0,
                )
            # diag chunk: notflag broadcast, trimmed to lower triangle (GPSIMD)
            nf_bcast = bass.AP(nfa.tensor, nfa.offset + i, [nfa.ap[0], [0, P]])
            nc.gpsimd.affine_select(
                out=ot[:, r0:w_cols], in_=nf_bcast, pattern=[[-1, P]],
                compare_op=Alu.is_ge, fill=0.0, base=0, channel_multiplier=1,
            )
            # force the diagonal to 1 (covers tt==1 rows) (DVE)
            nc.vector.tensor_max(ot[:, r0:w_cols], ot[:, r0:w_cols], diag128)

            nc.sync.dma_start(out=out_ap[0, r0 : r0 + P, 0:w_cols], in_=ot[:, 0:w_cols])
            nc.sync.dma_start(out=out_ap[1, r0 : r0 + P, 0:w_cols], in_=ot[:, 0:w_cols])

    nc.compile()
    return nc


def _get_program():
    if "nc" not in _CACHE:
        _CACHE["nc"] = _build_program()
    return _CACHE["nc"]


def _make_in_maps(input_tensor, gumbel_u, W, b):
    x = np.ascontiguousarray(np.asarray(input_tensor, dtype=np.float32))
    u = np.ascontiguousarray(np.asarray(gumbel_u, dtype=np.float32))
    w = np.ascontiguousarray(np.asarray(W, dtype=np.float32))
    bb = np.ascontiguousarray(np.asarray(b, dtype=np.float32))
    in_maps = []
    for c in range(N_CORES):
        bi = c // (N_CORES // B)
        in_maps.append({"x": x[bi], "u": u[bi], "w": w, "bb": bb})
    return in_maps


def _assemble(results):
    full = np.empty((B, H, S, S), dtype=np.float32)
    for c in range(N_CORES):
        bi = c // (N_CORES // B)
        q = c % (N_CORES // B)
        full[bi, 2 * q] = results[c]["out"][0]
        full[bi, 2 * q + 1] = results[c]["out"][1]
    return full


def kernel(input_tensor, token_types, gumbel_u, W, b, **_ignored):
    from concourse.bass_utils import run_bass_kernel_spmd

    nc = _get_program()
    in_maps = _make_in_maps(input_tensor, gumbel_u, W, b)
    res = run_bass_kernel_spmd(nc, in_maps, core_ids=list(range(N_CORES)))
    return _assemble(res.results)
